# revision 25
# baseline (speedup 1.0000x reference)
# Linear-attention layer (phi = elu+1) on 8 Trainium2 NeuronCores.
#
# Reference computation (per batch b):
#   q = x @ Wq + bq ; k = x @ Wk + bk ; v = x @ Wv + bv      [S, DM] each
#   kv[h] = phi(k_h)^T @ v_h          (sum over ALL of S)    [HD, HD]
#   attn_h = phi(q_h) @ kv[h]                                [S, HD]
#   out = attn @ Wo + bo                                     [S, DM]
#
# Sharding: batch-pair parallel. Core c owns HALF of batch c//2
# (sequence rows (c%2)*2048 : (c%2+1)*2048) -> 2048 rows per core, one
# batch per core. kv is a sum over sequence -> cores 2b and 2b+1 each
# compute a partial kv over their half and combine it with a single
# 128 KiB two-core AllReduce (replica groups [2b, 2b+1]). Unlike the
# previous sequence-parallel scheme (4 serialized 8-core AllReduces),
# each core participates in exactly ONE small pair collective, so
# cross-core rendezvous skew (tens of us, the dominant run-to-run
# variance) only couples neighbor pairs and the collective chain is
# gone. Every core finishes its own rows through attn + out_proj;
# output rows are disjoint across cores.
#
# phi(t) = elu(t) + 1 = exp(min(t, 0)) + relu(t)
#
# Numerics: x/W cast to bf16 on host, matmuls accumulate in fp32 PSUM,
# kv state reduced in bf16, output stored bf16 and cast to fp32 on host
# (validated ~4e-3 rel err vs fp32 ref; gate is 2e-2).
#
# Perf notes (see kernel_312.py.bak for the seq-parallel history):
#  - All inputs are host-repacked to match the SBUF tile layout exactly;
#    wk/wv/x are chunked per-kc ([P, 512] tiles) so the first matmul's
#    dependencies are 256 KiB, and the startup loads stream on three
#    queues (sync/gpsimd/scalar) in consumption order.
#  - Phase 4 consumes kv_bd (block-diag expansion built by GpSimd) with
#    full-K matmuls per (pair, quarter).
#  - phi computed as min(exp(t),1) + relu(t) == exp(min(t,0)) + relu(t):
#    2 ACT ops (both reading PSUM directly) + 1 fused DVE
#    scalar_tensor_tensor. The chip SW-throttles the PE clock to 13/16
#    (1.95 GHz) under sustained multi-engine load, so every bit of
#    non-PE activity is power that bids against the PE clock.
#  - kv-slot matmuls accumulate into DVE-pre-zeroed PSUM banks with
#    start=False (no PE pre-zeroing matmuls).
#  - wq rides the scalar queue behind wv (needed ~165 us); wo rides the
#    sync queue behind the AllReduce readback (needed ~250 us), keeping
#    4 MiB out of the startup DMA burst.

import numpy as np
import ml_dtypes

B, S, DM, H = 4, 4096, 1024, 16
HD = DM // H          # 64
N_CORES = 8
P = 128
R = 2048              # rows per core (half of one batch's sequence)
KC = DM // P          # 8 contraction chunks
SCH = R // P          # s-chunks per core (16)
NT_R = R // 512       # 512-wide row tiles (4)
ND = DM // 512        # 512-wide feature tiles (2)
KVB = (H // 2) * HD   # 512 columns of kv state (8 head-pairs x 64)

_cache = {}


def _build(has_bias):
    import concourse.mybir as mybir
    import concourse.tile as tile
    from concourse import bacc

    fp32 = mybir.dt.float32
    bf16 = mybir.dt.bfloat16
    AF = mybir.ActivationFunctionType
    ALU = mybir.AluOpType

    nc = bacc.Bacc("TRN2", target_bir_lowering=False, debug=False,
                   num_devices=N_CORES)

    # all DRAM inputs pre-packed on host to the exact SBUF tile layout
    x_d = nc.dram_tensor("x", [NT_R, P, KC * 512], bf16,
                         kind="ExternalInput").ap()
    wk_d = nc.dram_tensor("wk", [2, P, KC * 512], bf16,
                          kind="ExternalInput").ap()
    wv_d = nc.dram_tensor("wv", [2, P, KC * 512], bf16,
                          kind="ExternalInput").ap()
    wq_d = nc.dram_tensor("wq", [P, KC * DM], bf16, kind="ExternalInput").ap()
    wo_d = nc.dram_tensor("wo", [P, KC * DM], bf16, kind="ExternalInput").ap()
    bqc_d = nc.dram_tensor("bqc", [P, KC], fp32, kind="ExternalInput").ap()
    if has_bias:
        bk_d = nc.dram_tensor("bk2", [1, DM], bf16, kind="ExternalInput").ap()
        bv_d = nc.dram_tensor("bv2", [1, DM], bf16, kind="ExternalInput").ap()
        bo_d = nc.dram_tensor("bo2", [1, DM], bf16, kind="ExternalInput").ap()
    out_d = nc.dram_tensor("out", [R, DM], bf16, kind="ExternalOutput").ap()
    with tile.TileContext(nc) as tc:
        with (
            tc.tile_pool(name="big", bufs=1) as big,
            tc.tile_pool(name="stream", bufs=4) as stream,
            tc.tile_pool(name="tmp", bufs=2) as tmpp,
            tc.tile_pool(name="outp", bufs=3) as outp,
            tc.tile_pool(name="psum", bufs=6, space="PSUM") as psum,
            tc.tile_pool(name="kvps", bufs=2, space="PSUM") as kvps,
            tc.tile_pool(name="dram", bufs=1, space="DRAM") as dram,
        ):
            # ---------------- persistent tiles ----------------
            # wk/wv live only through phase 1 (own pool, closed after);
            # attnt is allocated afterwards and reuses their space.
            wkv_pool = tc.tile_pool(name="wkv", bufs=1)
            wkv = wkv_pool.__enter__()
            xt = [big.tile([P, KC, 512], bf16, tag=f"xt{rg}", name=f"xt{rg}")
                  for rg in range(NT_R)]          # x^T, split by row group
            wq = big.tile([P, KC, DM], bf16, tag="wq")
            wo = big.tile([P, KC, DM], bf16, tag="wo")
            wk = [wkv.tile([P, KC, 512], bf16, tag=f"wk{h}", name=f"wk{h}")
                  for h in range(2)]
            wv = [wkv.tile([P, KC, 512], bf16, tag=f"wv{h}", name=f"wv{h}")
                  for h in range(2)]
            phiq = big.tile([P, KC, R], bf16, tag="phiq")    # phi(q)^T
            # kv state: head-pair stacked on partitions (even head rows
            # 0:64, odd head rows 64:128); column slot pair*64
            kv_sb = big.tile([P, KVB], bf16, tag="kv")
            kv_rd = big.tile([P, KVB], bf16, tag="kvr")
            # block-diag expansion of kv_rd: per pair a [128,128] block
            # with kv_even at (0:64, 0:64), kv_odd at (64:128, 64:128), so
            # phase 4 runs full-K matmuls (no quadrant pipe-drain)
            kv_bd = big.tile([P, (H // 2) * P], bf16, tag="kvbd")
            bqc = big.tile([P, KC], fp32, tag="bqc")
            if has_bias:
                bk2 = big.tile([1, DM], bf16, tag="bk2")
                bv2 = big.tile([1, DM], bf16, tag="bv2")
                bo2 = big.tile([1, DM], bf16, tag="bo2")
            ones = big.tile([1, P], bf16, tag="ones")
            zrow = big.tile([1, 512], bf16, tag="zrow")

            kv_in = dram.tile([P, KVB], bf16, tag="kvi", name="kvi")
            kv_out = dram.tile([P, KVB], bf16, tag="kvo", name="kvo")

            def s512(n):
                return slice(n * 512, (n + 1) * 512)

            # ---------------- loads ----------------
            # Startup is DMA-bound: ~10 us DGE spin-up, then the two
            # hardware DGE queues (sync/scalar) share ~320 GB/s. The
            # gpsimd queue is the slow software DGE - smalls only.
            # Stream the startup-critical 5 MiB in exact phase-1
            # consumption order, wk0/xt0 interleaved kc-by-kc so the
            # first kp matmuls unblock progressively.
            for kc in range(KC):
                nc.sync.dma_start(wk[0][:, kc, :],
                                  wk_d[0][:, 512 * kc:512 * (kc + 1)])
                nc.scalar.dma_start(xt[0][:, kc, :],
                                    x_d[0][:, 512 * kc:512 * (kc + 1)])
            for hh in range(4):
                cs, ce = 2 * hh, 2 * hh + 2
                nc.sync.dma_start(wk[1][:, cs:ce, :],
                                  wk_d[1][:, 1024 * hh:1024 * (hh + 1)])
                nc.scalar.dma_start(wv[1][:, cs:ce, :],
                                    wv_d[1][:, 1024 * hh:1024 * (hh + 1)])
            for hh in range(4):
                cs, ce = 2 * hh, 2 * hh + 2
                nc.sync.dma_start(wv[0][:, cs:ce, :],
                                  wv_d[0][:, 1024 * hh:1024 * (hh + 1)])
                nc.scalar.dma_start(xt[1][:, cs:ce, :],
                                    x_d[1][:, 1024 * hh:1024 * (hh + 1)])
            nc.sync.dma_start(xt[2][:], x_d[2])
            nc.scalar.dma_start(xt[3][:], x_d[3])
            # wq is needed at ~165 us (phase 3): it follows the critical
            # set on the scalar queue, landing ~45 us
            nc.scalar.dma_start(wq[:], wq_d)
            nc.gpsimd.dma_start(bqc[:], bqc_d)
            if has_bias:
                nc.gpsimd.dma_start(bk2[:], bk_d)
                nc.gpsimd.dma_start(bv2[:], bv_d)
                nc.gpsimd.dma_start(bo2[:], bo_d)
            nc.gpsimd.memset(ones[:], 1.0)
            nc.gpsimd.memset(zrow[:], 0.0)
            # warm the ACT Exp/Relu LUTs during the launch/DMA window so the
            # first real phi ops skip the cold table load (~2us)
            wtile = big.tile([1, 8], bf16, tag="warm")
            nc.scalar.activation(out=wtile[:], in_=zrow[0:1, 0:8], func=AF.Exp)
            nc.scalar.activation(out=wtile[:], in_=zrow[0:1, 0:8], func=AF.Relu)
            nc.gpsimd.memset(kv_bd[:], 0.0)

            # ---------- phase 1: k/v projections + phi(k) + partial kv ----------
            kvp = [kvps.tile([P, 512], fp32, tag="kvp0", name="kvp0",
                             bufs=1),
                   kvps.tile([P, 512], fp32, tag="kvp1", name="kvp1",
                             bufs=1)]
            for sc in range(SCH):
                kch = stream.tile([P, DM], bf16, tag="kch")
                vch = stream.tile([P, DM], bf16, tag="vch")
                # k-projections (both halves) before v: matches weight
                # DMA arrival order at kernel start
                kp = [psum.tile([P, 512], fp32, tag="pp", name=f"kp{n}")
                      for n in range(ND)]
                vp = [psum.tile([P, 512], fp32, tag="pp", name=f"vp{n}")
                      for n in range(ND)]
                for n in range(ND):
                    for kc in range(KC):
                        nc.tensor.matmul(
                            kp[n][:],
                            lhsT=xt[sc // 4][:, kc,
                                             (sc % 4) * P:(sc % 4 + 1) * P],
                            rhs=wk[n][:, kc, :],
                            start=(kc == 0),
                            stop=(not has_bias and kc == KC - 1))
                    if has_bias:
                        nc.tensor.matmul(kp[n][:], lhsT=ones[:],
                                         rhs=bk2[:, s512(n)],
                                         start=False, stop=True)
                for n in (1, 0):
                    for kc in range(KC):
                        nc.tensor.matmul(
                            vp[n][:],
                            lhsT=xt[sc // 4][:, kc,
                                             (sc % 4) * P:(sc % 4 + 1) * P],
                            rhs=wv[n][:, kc, :],
                            start=(kc == 0),
                            stop=(not has_bias and kc == KC - 1))
                    if has_bias:
                        nc.tensor.matmul(vp[n][:], lhsT=ones[:],
                                         rhs=bv2[:, s512(n)],
                                         start=False, stop=True)
                # phi(k) = min(exp(k),1) + relu(k); ops placed so every
                # kv-slot input is ready just before the PE needs it:
                # ACT: exp0, exp1, relu0, vch0-half / DVE: relu1, stt1,
                # vch1, stt0, vch0-half. kv pairs 4-7 (n=1 inputs) are
                # emitted before pairs 0-3 to match.
                et = [tmpp.tile([P, 512], bf16, tag="e", name=f"et{n}")
                      for n in range(ND)]
                rt = [tmpp.tile([P, 512], bf16, tag="r", name=f"rt{n}")
                      for n in range(ND)]
                nc.scalar.activation(out=et[0][:], in_=kp[0][:],
                                     func=AF.Exp)
                nc.scalar.activation(out=et[1][:], in_=kp[1][:],
                                     func=AF.Exp)
                nc.vector.tensor_scalar_max(out=rt[1][:], in0=kp[1][:],
                                            scalar1=0.0)
                nc.vector.scalar_tensor_tensor(
                    out=kch[:, s512(1)], in0=et[1][:], scalar=1.0,
                    in1=rt[1][:], op0=ALU.min, op1=ALU.add)
                nc.scalar.activation(out=rt[0][:], in_=kp[0][:],
                                     func=AF.Relu)
                nc.vector.tensor_copy(out=vch[:, s512(1)], in_=vp[1][:])
                nc.vector.scalar_tensor_tensor(
                    out=kch[:, s512(0)], in0=et[0][:], scalar=1.0,
                    in1=rt[0][:], op0=ALU.min, op1=ALU.add)
                nc.vector.tensor_copy(out=vch[:, 0:256],
                                      in_=vp[0][:, 0:256])
                nc.scalar.activation(out=vch[:, 256:512],
                                     in_=vp[0][:, 256:512], func=AF.Copy)
                if sc == 0:
                    # DVE-zero the accumulator banks (keeps zeroing off
                    # the PE). With data=0, PSUM has_written semantics
                    # don't matter: overwrite-with-result and
                    # accumulate-onto-zero are identical, so the slot
                    # matmuls can all run start=False.
                    for j in (0, 1):
                        nc.vector.memset(kvp[j][:], 0.0)
                for pr in (4, 5, 6, 7, 0, 1, 2, 3):
                    j, col = pr // 4, (pr % 4) * P
                    # full pair x pair cross-product; diagonal 64x64
                    # blocks are the two heads' TRANSPOSED kv states
                    # (v^T phi(k), i.e. kv^T), which is what the
                    # M = kv_bd @ Wo fold consumes as lhsT
                    nc.tensor.matmul(
                        kvp[j][:, col:col + P],
                        lhsT=vch[:, pr * P:(pr + 1) * P],
                        rhs=kch[:, pr * P:(pr + 1) * P],
                        start=False,
                        stop=(sc == SCH - 1 and pr % 4 == 3),
                        skip_group_check=True)
            for h in range(H):
                pr = h // 2
                j, col = pr // 4, (pr % 4) * P + (h % 2) * HD
                rows = slice((h % 2) * HD, (h % 2 + 1) * HD)
                slot = pr * HD
                nc.vector.tensor_copy(
                    out=kv_sb[rows, slot:slot + HD],
                    in_=kvp[j][rows, col:col + HD])
            # wk/wv dead from here; free their SBUF for M = kv_bd @ Wo
            wkv_pool.__exit__(None, None, None)
            m_sb = big.tile([P, KC, DM], bf16, tag="msb")

            # single two-core AllReduce (128 KiB) with the pair neighbor:
            # fires at end of phase 1 (~150 us), needed at ~250 us.
            # Emitted BEFORE phase 3 so the deferred wo load (sync queue,
            # behind the readback) is a write-before-read in program order.
            nc.gpsimd.dma_start(kv_in[:], kv_sb[:])
            nc.gpsimd.collective_compute(
                "AllReduce",
                mybir.AluOpType.add,
                replica_groups=[[2 * b, 2 * b + 1] for b in range(B)],
                ins=[kv_in.opt()],
                outs=[kv_out.opt()],
            )
            nc.sync.dma_start(kv_rd[:], kv_out[:])
            nc.sync.dma_start(wo[:], wo_d)

            # block-diag expansion on the (otherwise idle) GpSimd engine
            for h in range(H):
                pr = h // 2
                rows = slice((h % 2) * HD, (h % 2 + 1) * HD)
                bdc = pr * P + (h % 2) * HD
                slot = pr * HD
                nc.gpsimd.tensor_copy(
                    out=kv_bd[rows, bdc:bdc + HD],
                    in_=kv_rd[rows, slot:slot + HD])

            # ---------- phase 3: q^T projection + phi ----------
            for m in range(KC):
                for nt in range(NT_R):
                    qps = psum.tile([P, 512], fp32, tag="pp")
                    for kc in range(KC):
                        nc.tensor.matmul(
                            qps[:],
                            lhsT=wq[:, kc, m * P:(m + 1) * P],
                            rhs=xt[nt][:, kc, :],
                            start=(kc == 0), stop=(kc == KC - 1))
                    # phi(q+bq) = min(exp(q+bq),1) + relu(q+bq); relu on DVE
                    # so ACT (exp) and DVE (relu+combine) stay balanced
                    et = tmpp.tile([P, 512], bf16, tag="e")
                    nc.scalar.activation(out=et[:], in_=qps[:], func=AF.Exp,
                                         bias=bqc[:, m:m + 1], scale=1.0)
                    rt = tmpp.tile([P, 512], bf16, tag="r")
                    nc.vector.tensor_scalar(out=rt[:], in0=qps[:],
                                            scalar1=bqc[:, m:m + 1],
                                            scalar2=0.0,
                                            op0=ALU.add, op1=ALU.max)
                    nc.vector.scalar_tensor_tensor(
                        out=phiq[:, m, s512(nt)], in0=et[:], scalar=1.0,
                        in1=rt[:], op0=ALU.min, op1=ALU.add)

            # ---------- phase 4: M = kv_bd @ Wo ----------
            # kv_bd holds block-diagonal kv^T, so M's block-row pr is
            # kv_block @ Wo[pr*128:(pr+1)*128, :] = one full-K matmul
            # (lhsT = kv^T block). 16 matmuls of N=512 (8192 cycles) vs
            # the former attn materialization (16384 cycles + 32 copies).
            for pr in range(H // 2):
                for n in range(ND):
                    mps = psum.tile([P, 512], fp32, tag="pp", name="mps")
                    nc.tensor.matmul(
                        mps[:],
                        lhsT=kv_bd[:, pr * P:(pr + 1) * P],
                        rhs=wo[:, pr, s512(n)],
                        start=True, stop=True)
                    if n:
                        nc.vector.tensor_copy(
                            out=m_sb[:, pr, s512(n)], in_=mps[:])
                    else:
                        nc.scalar.activation(
                            out=m_sb[:, pr, s512(n)], in_=mps[:],
                            func=AF.Copy)

            # ---------- phase 5: out = phi(q) @ M + bo ----------
            def p5_group(g, n):
                ops = psum.tile([P, 512], fp32, tag="pp", name="ops")
                for kc in range(KC):
                    nc.tensor.matmul(
                        ops[:], lhsT=phiq[:, kc, g * P:(g + 1) * P],
                        rhs=m_sb[:, kc, s512(n)],
                        start=(kc == 0),
                        stop=(not has_bias and kc == KC - 1))
                if has_bias:
                    nc.tensor.matmul(ops[:], lhsT=ones[:],
                                     rhs=bo2[:, s512(n)],
                                     start=False, stop=True)
                osb = outp.tile([P, 512], bf16, tag="osb")
                nc.vector.tensor_copy(out=osb[:, 0:256], in_=ops[:, 0:256])
                nc.scalar.activation(out=osb[:, 256:512],
                                     in_=ops[:, 256:512], func=AF.Copy)
                nc.sync.dma_start(out_d[g * P:(g + 1) * P, s512(n)], osb[:])

            for g in range(SCH):
                for n in range(ND):
                    p5_group(g, n)

    nc.compile()
    return nc


def _get_nc(has_bias):
    key = ("nc", has_bias)
    if key not in _cache:
        _cache[key] = _build(has_bias)
    return _cache[key]


def _has_bias(inputs):
    return any(np.any(np.asarray(inputs[k], np.float32))
               for k in ("bk", "bv", "bo"))


def _pack_w_halves(w):
    # [2, P, KC*512]: w2[h, p, c*512+j] = W[c*128+p, h*512+j]
    return np.ascontiguousarray(
        w.reshape(KC, P, 2, 512).transpose(2, 1, 0, 3).reshape(2, P, KC * 512))


def _pack_w_full(w):
    # [P, KC*DM]: wf[p, c*DM+j] = W[c*128+p, j]
    return np.ascontiguousarray(
        w.reshape(KC, P, DM).transpose(1, 0, 2).reshape(P, KC * DM))


def _make_in_maps(inputs, has_bias):
    bf16 = ml_dtypes.bfloat16
    x = np.asarray(inputs["x"], dtype=np.float32)
    ws = {k: np.asarray(inputs[k], np.float32).astype(bf16)
          for k in ("Wq", "Wk", "Wv", "Wo")}
    wk2 = _pack_w_halves(ws["Wk"])
    wv2 = _pack_w_halves(ws["Wv"])
    wqf = _pack_w_full(ws["Wq"])
    wof = _pack_w_full(ws["Wo"])
    bq = np.asarray(inputs["bq"], np.float32)
    bqc = np.ascontiguousarray(bq.reshape(KC, P).T.astype(np.float32))
    brow = {k: np.ascontiguousarray(
                np.asarray(inputs[k], np.float32).astype(bf16).reshape(1, DM))
            for k in ("bk", "bv", "bo")}
    xb = x.astype(bf16)
    in_maps = []
    for c in range(N_CORES):
        # core c: batch c//2, sequence rows (c%2)*R : (c%2+1)*R
        # xt[rg][p][c_*512+j] = x_core^T[c_*128+p, rg*512+j]
        xsT = xb[c // 2, (c % 2) * R:(c % 2 + 1) * R, :].T
        xs = np.ascontiguousarray(
            xsT.reshape(KC, P, NT_R, 512).transpose(2, 1, 0, 3)
               .reshape(NT_R, P, KC * 512))
        m = {
            "x": xs,
            "wq": wqf, "wk": wk2, "wv": wv2, "wo": wof,
            "bqc": bqc,
        }
        if has_bias:
            m.update({"bk2": brow["bk"], "bv2": brow["bv"],
                      "bo2": brow["bo"]})
        in_maps.append(m)
    return in_maps


def _run(inputs, **kw):
    from concourse import bass_utils
    hb = _has_bias(inputs)
    nc = _get_nc(hb)
    in_maps = _make_in_maps(inputs, hb)
    res = bass_utils.run_bass_kernel_spmd(
        nc, in_maps, core_ids=list(range(N_CORES)), **kw)
    out = np.empty((B, S, DM), np.float32)
    for c in range(N_CORES):
        out[c // 2, (c % 2) * R:(c % 2 + 1) * R, :] = (
            res.results[c]["out"].astype(np.float32))
    return out, res


def kernel(**inputs) -> np.ndarray:
    out, _ = _run(inputs)
    return out


# revision 28
# speedup vs baseline: 1.0171x; 1.0171x over previous
# Linear-attention layer (phi = elu+1) on 8 Trainium2 NeuronCores.
#
# Reference computation (per batch b):
#   q = x @ Wq + bq ; k = x @ Wk + bk ; v = x @ Wv + bv      [S, DM] each
#   kv[h] = phi(k_h)^T @ v_h          (sum over ALL of S)    [HD, HD]
#   attn_h = phi(q_h) @ kv[h]                                [S, HD]
#   out = attn @ Wo + bo                                     [S, DM]
#
# Sharding: batch-pair parallel. Core c owns HALF of batch c//2
# (sequence rows (c%2)*2048 : (c%2+1)*2048) -> 2048 rows per core, one
# batch per core. kv is a sum over sequence -> cores 2b and 2b+1 each
# compute a partial kv over their half and combine it with a single
# 128 KiB two-core AllReduce (replica groups [2b, 2b+1]). Unlike the
# previous sequence-parallel scheme (4 serialized 8-core AllReduces),
# each core participates in exactly ONE small pair collective, so
# cross-core rendezvous skew (tens of us, the dominant run-to-run
# variance) only couples neighbor pairs and the collective chain is
# gone. Every core finishes its own rows through attn + out_proj;
# output rows are disjoint across cores.
#
# phi(t) = elu(t) + 1 = exp(min(t, 0)) + relu(t)
#
# Numerics: x/W cast to bf16 on host, matmuls accumulate in fp32 PSUM,
# kv state reduced in bf16, output stored bf16 and cast to fp32 on host
# (validated ~4e-3 rel err vs fp32 ref; gate is 2e-2).
#
# Perf notes (see kernel_312.py.bak for the seq-parallel history):
#  - All inputs are host-repacked to match the SBUF tile layout exactly;
#    wk/wv/x are chunked per-kc ([P, 512] tiles) so the first matmul's
#    dependencies are 256 KiB, and the startup loads stream on three
#    queues (sync/gpsimd/scalar) in consumption order.
#  - Phase 4 consumes kv_bd (block-diag expansion built by GpSimd) with
#    full-K matmuls per (pair, quarter).
#  - phi computed as min(exp(t),1) + relu(t) == exp(min(t,0)) + relu(t):
#    2 ACT ops (both reading PSUM directly) + 1 fused DVE
#    scalar_tensor_tensor. The chip SW-throttles the PE clock to 13/16
#    (1.95 GHz) under sustained multi-engine load, so every bit of
#    non-PE activity is power that bids against the PE clock.
#  - kv-slot matmuls accumulate into DVE-pre-zeroed PSUM banks with
#    start=False (no PE pre-zeroing matmuls).
#  - Phase 4 folds the kv state into the out-projection: M = kv_bd @ Wo
#    (16 full-K matmuls) and out = phi(q) @ M, replacing the attn
#    materialization (half the PE cycles there, 16 fewer copies).
#  - wq/wo stream last behind the startup-critical set (~45-55 us).

import numpy as np
import ml_dtypes

B, S, DM, H = 4, 4096, 1024, 16
HD = DM // H          # 64
N_CORES = 8
P = 128
R = 2048              # rows per core (half of one batch's sequence)
KC = DM // P          # 8 contraction chunks
SCH = R // P          # s-chunks per core (16)
NT_R = R // 512       # 512-wide row tiles (4)
ND = DM // 512        # 512-wide feature tiles (2)
KVB = (H // 2) * HD   # 512 columns of kv state (8 head-pairs x 64)

_cache = {}


def _build(has_bias):
    import concourse.mybir as mybir
    import concourse.tile as tile
    from concourse import bacc

    fp32 = mybir.dt.float32
    bf16 = mybir.dt.bfloat16
    AF = mybir.ActivationFunctionType
    ALU = mybir.AluOpType

    nc = bacc.Bacc("TRN2", target_bir_lowering=False, debug=False,
                   num_devices=N_CORES)

    # all DRAM inputs pre-packed on host to the exact SBUF tile layout
    x_d = nc.dram_tensor("x", [NT_R, P, KC * 512], bf16,
                         kind="ExternalInput").ap()
    wk_d = nc.dram_tensor("wk", [2, P, KC * 512], bf16,
                          kind="ExternalInput").ap()
    wv_d = nc.dram_tensor("wv", [2, P, KC * 512], bf16,
                          kind="ExternalInput").ap()
    wq_d = nc.dram_tensor("wq", [P, KC * DM], bf16, kind="ExternalInput").ap()
    wo_d = nc.dram_tensor("wo", [P, KC * DM], bf16, kind="ExternalInput").ap()
    bqc_d = nc.dram_tensor("bqc", [P, KC], fp32, kind="ExternalInput").ap()
    if has_bias:
        bk_d = nc.dram_tensor("bk2", [1, DM], bf16, kind="ExternalInput").ap()
        bv_d = nc.dram_tensor("bv2", [1, DM], bf16, kind="ExternalInput").ap()
        bo_d = nc.dram_tensor("bo2", [1, DM], bf16, kind="ExternalInput").ap()
    out_d = nc.dram_tensor("out", [R, DM], bf16, kind="ExternalOutput").ap()
    with tile.TileContext(nc) as tc:
        with (
            tc.tile_pool(name="big", bufs=1) as big,
            tc.tile_pool(name="stream", bufs=4) as stream,
            tc.tile_pool(name="tmp", bufs=2) as tmpp,
            tc.tile_pool(name="outp", bufs=3) as outp,
            tc.tile_pool(name="psum", bufs=6, space="PSUM") as psum,
            tc.tile_pool(name="kvps", bufs=2, space="PSUM") as kvps,
            tc.tile_pool(name="dram", bufs=1, space="DRAM") as dram,
        ):
            # ---------------- persistent tiles ----------------
            # wk/wv live only through phase 1 (own pool, closed after);
            # m_sb is allocated afterwards and reuses their space.
            wkv_pool = tc.tile_pool(name="wkv", bufs=1)
            wkv = wkv_pool.__enter__()
            xt = [big.tile([P, KC, 512], bf16, tag=f"xt{rg}", name=f"xt{rg}")
                  for rg in range(NT_R)]          # x^T, split by row group
            wq = big.tile([P, KC, DM], bf16, tag="wq")
            wo = big.tile([P, KC, DM], bf16, tag="wo")
            wk = [wkv.tile([P, KC, 512], bf16, tag=f"wk{h}", name=f"wk{h}")
                  for h in range(2)]
            wv = [wkv.tile([P, KC, 512], bf16, tag=f"wv{h}", name=f"wv{h}")
                  for h in range(2)]
            phiq = big.tile([P, KC, R], bf16, tag="phiq")    # phi(q)^T
            # kv state: head-pair stacked on partitions (even head rows
            # 0:64, odd head rows 64:128); column slot pair*64
            kv_sb = big.tile([P, KVB], bf16, tag="kv")
            kv_rd = big.tile([P, KVB], bf16, tag="kvr")
            # block-diag expansion of kv_rd: per pair a [128,128] block
            # with kv_even at (0:64, 0:64), kv_odd at (64:128, 64:128), so
            # phase 4 runs full-K matmuls (no quadrant pipe-drain)
            kv_bd = big.tile([P, (H // 2) * P], bf16, tag="kvbd")
            bqc = big.tile([P, KC], fp32, tag="bqc")
            if has_bias:
                bk2 = big.tile([1, DM], bf16, tag="bk2")
                bv2 = big.tile([1, DM], bf16, tag="bv2")
                bo2 = big.tile([1, DM], bf16, tag="bo2")
            ones = big.tile([1, P], bf16, tag="ones")
            zrow = big.tile([1, 512], bf16, tag="zrow")

            kv_in = dram.tile([P, KVB], bf16, tag="kvi", name="kvi")
            kv_out = dram.tile([P, KVB], bf16, tag="kvo", name="kvo")

            def s512(n):
                return slice(n * 512, (n + 1) * 512)

            # ---------------- loads ----------------
            # Startup is DMA-bound: ~10 us DGE spin-up, then the two
            # hardware DGE queues (sync/scalar) share ~320 GB/s. The
            # gpsimd queue is the slow software DGE - smalls only.
            # Stream the startup-critical 5 MiB in exact phase-1
            # consumption order, wk0/xt0 interleaved kc-by-kc so the
            # first kp matmuls unblock progressively.
            for kc in range(KC):
                nc.sync.dma_start(wk[0][:, kc, :],
                                  wk_d[0][:, 512 * kc:512 * (kc + 1)])
                nc.scalar.dma_start(xt[0][:, kc, :],
                                    x_d[0][:, 512 * kc:512 * (kc + 1)])
            for hh in range(4):
                cs, ce = 2 * hh, 2 * hh + 2
                nc.sync.dma_start(wk[1][:, cs:ce, :],
                                  wk_d[1][:, 1024 * hh:1024 * (hh + 1)])
                nc.scalar.dma_start(wv[1][:, cs:ce, :],
                                    wv_d[1][:, 1024 * hh:1024 * (hh + 1)])
            for hh in range(4):
                cs, ce = 2 * hh, 2 * hh + 2
                nc.sync.dma_start(wv[0][:, cs:ce, :],
                                  wv_d[0][:, 1024 * hh:1024 * (hh + 1)])
                nc.scalar.dma_start(xt[1][:, cs:ce, :],
                                    x_d[1][:, 1024 * hh:1024 * (hh + 1)])
            nc.sync.dma_start(xt[2][:], x_d[2])
            nc.scalar.dma_start(xt[3][:], x_d[3])
            # wq (phase 3, ~150 us) and wo (phase 4 M-fold, ~205 us) ride
            # last behind the startup-critical set, landing ~45-55 us;
            # with the pair-collective scheme the power throttle is gone,
            # so there is no reason to defer them behind the readback
            # (which would put wo's arrival ~15 us before its use)
            nc.scalar.dma_start(wq[:], wq_d)
            nc.sync.dma_start(wo[:], wo_d)
            nc.gpsimd.dma_start(bqc[:], bqc_d)
            if has_bias:
                nc.gpsimd.dma_start(bk2[:], bk_d)
                nc.gpsimd.dma_start(bv2[:], bv_d)
                nc.gpsimd.dma_start(bo2[:], bo_d)
            nc.gpsimd.memset(ones[:], 1.0)
            nc.gpsimd.memset(zrow[:], 0.0)
            # warm the ACT Exp/Relu LUTs during the launch/DMA window so the
            # first real phi ops skip the cold table load (~2us)
            wtile = big.tile([1, 8], bf16, tag="warm")
            nc.scalar.activation(out=wtile[:], in_=zrow[0:1, 0:8], func=AF.Exp)
            nc.scalar.activation(out=wtile[:], in_=zrow[0:1, 0:8], func=AF.Relu)
            nc.gpsimd.memset(kv_bd[:], 0.0)

            # ---------- phase 1: k/v projections + phi(k) + partial kv ----------
            kvp = [kvps.tile([P, 512], fp32, tag="kvp0", name="kvp0",
                             bufs=1),
                   kvps.tile([P, 512], fp32, tag="kvp1", name="kvp1",
                             bufs=1)]
            for sc in range(SCH):
                kch = stream.tile([P, DM], bf16, tag="kch")
                vch = stream.tile([P, DM], bf16, tag="vch")
                # k-projections (both halves) before v: matches weight
                # DMA arrival order at kernel start
                kp = [psum.tile([P, 512], fp32, tag="pp", name=f"kp{n}")
                      for n in range(ND)]
                vp = [psum.tile([P, 512], fp32, tag="pp", name=f"vp{n}")
                      for n in range(ND)]
                for n in range(ND):
                    for kc in range(KC):
                        nc.tensor.matmul(
                            kp[n][:],
                            lhsT=xt[sc // 4][:, kc,
                                             (sc % 4) * P:(sc % 4 + 1) * P],
                            rhs=wk[n][:, kc, :],
                            start=(kc == 0),
                            stop=(not has_bias and kc == KC - 1))
                    if has_bias:
                        nc.tensor.matmul(kp[n][:], lhsT=ones[:],
                                         rhs=bk2[:, s512(n)],
                                         start=False, stop=True)
                for n in (1, 0):
                    for kc in range(KC):
                        nc.tensor.matmul(
                            vp[n][:],
                            lhsT=xt[sc // 4][:, kc,
                                             (sc % 4) * P:(sc % 4 + 1) * P],
                            rhs=wv[n][:, kc, :],
                            start=(kc == 0),
                            stop=(not has_bias and kc == KC - 1))
                    if has_bias:
                        nc.tensor.matmul(vp[n][:], lhsT=ones[:],
                                         rhs=bv2[:, s512(n)],
                                         start=False, stop=True)
                # phi(k) = min(exp(k),1) + relu(k); ops placed so every
                # kv-slot input is ready just before the PE needs it:
                # ACT: exp0, exp1, relu0, vch0-half / DVE: relu1, stt1,
                # vch1, stt0, vch0-half. kv pairs 4-7 (n=1 inputs) are
                # emitted before pairs 0-3 to match.
                et = [tmpp.tile([P, 512], bf16, tag="e", name=f"et{n}")
                      for n in range(ND)]
                rt = [tmpp.tile([P, 512], bf16, tag="r", name=f"rt{n}")
                      for n in range(ND)]
                nc.scalar.activation(out=et[0][:], in_=kp[0][:],
                                     func=AF.Exp)
                nc.scalar.activation(out=et[1][:], in_=kp[1][:],
                                     func=AF.Exp)
                nc.vector.tensor_scalar_max(out=rt[1][:], in0=kp[1][:],
                                            scalar1=0.0)
                nc.vector.scalar_tensor_tensor(
                    out=kch[:, s512(1)], in0=et[1][:], scalar=1.0,
                    in1=rt[1][:], op0=ALU.min, op1=ALU.add)
                nc.scalar.activation(out=rt[0][:], in_=kp[0][:],
                                     func=AF.Relu)
                nc.vector.tensor_copy(out=vch[:, s512(1)], in_=vp[1][:])
                nc.vector.scalar_tensor_tensor(
                    out=kch[:, s512(0)], in0=et[0][:], scalar=1.0,
                    in1=rt[0][:], op0=ALU.min, op1=ALU.add)
                nc.vector.tensor_copy(out=vch[:, 0:256],
                                      in_=vp[0][:, 0:256])
                nc.scalar.activation(out=vch[:, 256:512],
                                     in_=vp[0][:, 256:512], func=AF.Copy)
                if sc == 0:
                    # DVE-zero the accumulator banks (keeps zeroing off
                    # the PE). With data=0, PSUM has_written semantics
                    # don't matter: overwrite-with-result and
                    # accumulate-onto-zero are identical, so the slot
                    # matmuls can all run start=False.
                    for j in (0, 1):
                        nc.vector.memset(kvp[j][:], 0.0)
                for pr in (4, 5, 6, 7, 0, 1, 2, 3):
                    j, col = pr // 4, (pr % 4) * P
                    # full pair x pair cross-product; diagonal 64x64
                    # blocks are the two heads' TRANSPOSED kv states
                    # (v^T phi(k), i.e. kv^T), which is what the
                    # M = kv_bd @ Wo fold consumes as lhsT
                    nc.tensor.matmul(
                        kvp[j][:, col:col + P],
                        lhsT=vch[:, pr * P:(pr + 1) * P],
                        rhs=kch[:, pr * P:(pr + 1) * P],
                        start=False,
                        stop=(sc == SCH - 1 and pr % 4 == 3),
                        skip_group_check=True)
            for h in range(H):
                pr = h // 2
                j, col = pr // 4, (pr % 4) * P + (h % 2) * HD
                rows = slice((h % 2) * HD, (h % 2 + 1) * HD)
                slot = pr * HD
                nc.vector.tensor_copy(
                    out=kv_sb[rows, slot:slot + HD],
                    in_=kvp[j][rows, col:col + HD])
            # wk/wv dead from here; free their SBUF for M = kv_bd @ Wo
            wkv_pool.__exit__(None, None, None)
            m_sb = big.tile([P, KC, DM], bf16, tag="msb")

            # single two-core AllReduce (128 KiB) with the pair neighbor:
            # fires at end of phase 1 (~150 us), needed at ~250 us.
            # Emitted BEFORE phase 3 so the deferred wo load (sync queue,
            # behind the readback) is a write-before-read in program order.
            nc.gpsimd.dma_start(kv_in[:], kv_sb[:])
            nc.gpsimd.collective_compute(
                "AllReduce",
                mybir.AluOpType.add,
                replica_groups=[[2 * b, 2 * b + 1] for b in range(B)],
                ins=[kv_in.opt()],
                outs=[kv_out.opt()],
            )
            nc.sync.dma_start(kv_rd[:], kv_out[:])

            # block-diag expansion on the (otherwise idle) GpSimd engine
            for h in range(H):
                pr = h // 2
                rows = slice((h % 2) * HD, (h % 2 + 1) * HD)
                bdc = pr * P + (h % 2) * HD
                slot = pr * HD
                nc.gpsimd.tensor_copy(
                    out=kv_bd[rows, bdc:bdc + HD],
                    in_=kv_rd[rows, slot:slot + HD])

            # ---------- phase 3: q^T projection + phi ----------
            for m in range(KC):
                for nt in range(NT_R):
                    qps = psum.tile([P, 512], fp32, tag="pp")
                    for kc in range(KC):
                        nc.tensor.matmul(
                            qps[:],
                            lhsT=wq[:, kc, m * P:(m + 1) * P],
                            rhs=xt[nt][:, kc, :],
                            start=(kc == 0), stop=(kc == KC - 1))
                    # phi(q+bq) = min(exp(q+bq),1) + relu(q+bq); relu on DVE
                    # so ACT (exp) and DVE (relu+combine) stay balanced
                    et = tmpp.tile([P, 512], bf16, tag="e")
                    nc.scalar.activation(out=et[:], in_=qps[:], func=AF.Exp,
                                         bias=bqc[:, m:m + 1], scale=1.0)
                    rt = tmpp.tile([P, 512], bf16, tag="r")
                    nc.vector.tensor_scalar(out=rt[:], in0=qps[:],
                                            scalar1=bqc[:, m:m + 1],
                                            scalar2=0.0,
                                            op0=ALU.add, op1=ALU.max)
                    nc.vector.scalar_tensor_tensor(
                        out=phiq[:, m, s512(nt)], in0=et[:], scalar=1.0,
                        in1=rt[:], op0=ALU.min, op1=ALU.add)

            # ---------- phase 4: M = kv_bd @ Wo ----------
            # kv_bd holds block-diagonal kv^T, so M's block-row pr is
            # kv_block @ Wo[pr*128:(pr+1)*128, :] = one full-K matmul
            # (lhsT = kv^T block). 16 matmuls of N=512 (8192 cycles) vs
            # the former attn materialization (16384 cycles + 32 copies).
            for pr in range(H // 2):
                for n in range(ND):
                    mps = psum.tile([P, 512], fp32, tag="pp", name="mps")
                    nc.tensor.matmul(
                        mps[:],
                        lhsT=kv_bd[:, pr * P:(pr + 1) * P],
                        rhs=wo[:, pr, s512(n)],
                        start=True, stop=True)
                    if n:
                        nc.vector.tensor_copy(
                            out=m_sb[:, pr, s512(n)], in_=mps[:])
                    else:
                        nc.scalar.activation(
                            out=m_sb[:, pr, s512(n)], in_=mps[:],
                            func=AF.Copy)

            # ---------- phase 5: out = phi(q) @ M + bo ----------
            def p5_group(g, n):
                ops = psum.tile([P, 512], fp32, tag="pp", name="ops")
                for kc in range(KC):
                    nc.tensor.matmul(
                        ops[:], lhsT=phiq[:, kc, g * P:(g + 1) * P],
                        rhs=m_sb[:, kc, s512(n)],
                        start=(kc == 0),
                        stop=(not has_bias and kc == KC - 1))
                if has_bias:
                    nc.tensor.matmul(ops[:], lhsT=ones[:],
                                     rhs=bo2[:, s512(n)],
                                     start=False, stop=True)
                osb = outp.tile([P, 512], bf16, tag="osb")
                nc.vector.tensor_copy(out=osb[:, 0:256], in_=ops[:, 0:256])
                nc.scalar.activation(out=osb[:, 256:512],
                                     in_=ops[:, 256:512], func=AF.Copy)
                nc.sync.dma_start(out_d[g * P:(g + 1) * P, s512(n)], osb[:])

            for g in range(SCH):
                for n in range(ND):
                    p5_group(g, n)

    nc.compile()
    return nc


def _get_nc(has_bias):
    key = ("nc", has_bias)
    if key not in _cache:
        _cache[key] = _build(has_bias)
    return _cache[key]


def _has_bias(inputs):
    return any(np.any(np.asarray(inputs[k], np.float32))
               for k in ("bk", "bv", "bo"))


def _pack_w_halves(w):
    # [2, P, KC*512]: w2[h, p, c*512+j] = W[c*128+p, h*512+j]
    return np.ascontiguousarray(
        w.reshape(KC, P, 2, 512).transpose(2, 1, 0, 3).reshape(2, P, KC * 512))


def _pack_w_full(w):
    # [P, KC*DM]: wf[p, c*DM+j] = W[c*128+p, j]
    return np.ascontiguousarray(
        w.reshape(KC, P, DM).transpose(1, 0, 2).reshape(P, KC * DM))


def _make_in_maps(inputs, has_bias):
    bf16 = ml_dtypes.bfloat16
    x = np.asarray(inputs["x"], dtype=np.float32)
    ws = {k: np.asarray(inputs[k], np.float32).astype(bf16)
          for k in ("Wq", "Wk", "Wv", "Wo")}
    wk2 = _pack_w_halves(ws["Wk"])
    wv2 = _pack_w_halves(ws["Wv"])
    wqf = _pack_w_full(ws["Wq"])
    wof = _pack_w_full(ws["Wo"])
    bq = np.asarray(inputs["bq"], np.float32)
    bqc = np.ascontiguousarray(bq.reshape(KC, P).T.astype(np.float32))
    brow = {k: np.ascontiguousarray(
                np.asarray(inputs[k], np.float32).astype(bf16).reshape(1, DM))
            for k in ("bk", "bv", "bo")}
    xb = x.astype(bf16)
    in_maps = []
    for c in range(N_CORES):
        # core c: batch c//2, sequence rows (c%2)*R : (c%2+1)*R
        # xt[rg][p][c_*512+j] = x_core^T[c_*128+p, rg*512+j]
        xsT = xb[c // 2, (c % 2) * R:(c % 2 + 1) * R, :].T
        xs = np.ascontiguousarray(
            xsT.reshape(KC, P, NT_R, 512).transpose(2, 1, 0, 3)
               .reshape(NT_R, P, KC * 512))
        m = {
            "x": xs,
            "wq": wqf, "wk": wk2, "wv": wv2, "wo": wof,
            "bqc": bqc,
        }
        if has_bias:
            m.update({"bk2": brow["bk"], "bv2": brow["bv"],
                      "bo2": brow["bo"]})
        in_maps.append(m)
    return in_maps


def _run(inputs, **kw):
    from concourse import bass_utils
    hb = _has_bias(inputs)
    nc = _get_nc(hb)
    in_maps = _make_in_maps(inputs, hb)
    res = bass_utils.run_bass_kernel_spmd(
        nc, in_maps, core_ids=list(range(N_CORES)), **kw)
    out = np.empty((B, S, DM), np.float32)
    for c in range(N_CORES):
        out[c // 2, (c % 2) * R:(c % 2 + 1) * R, :] = (
            res.results[c]["out"].astype(np.float32))
    return out, res


def kernel(**inputs) -> np.ndarray:
    out, _ = _run(inputs)
    return out
